# revision 22
# baseline (speedup 1.0000x reference)
"""Trainium2 Bass kernel for single-head causal attention.

  q = Xq @ Wq.T + bq ; k = Xk @ Wk.T + bk ; v = Xv @ Wv.T + bv
  out = softmax((q k^T + causal_mask)/sqrt(D)) @ v

Shapes: B=4, S=2048, D=1024, fp32 in/out.  8 NeuronCores, SPMD.

Sharding: core c handles batch b = c//2, parity h = c%2.  S splits into 16
q-tiles of 128; causal attention for q-tile g touches k-tiles 0..g.  Core
parity h owns q-tiles g = 2j + h (j = 0..7), and slot j statically
processes 2j+2 k-tiles on BOTH parities (identical SPMD program); the
h=0 core's last k-tile per slot is fully masked, so the per-core causal
mask is one static [128, 256] tile covering the last two k-tiles.

Compute (all bf16 matmuls, f32 psum):
  - K^T, Q^T projected to [e-part, s] layout, V to [s-part, d]; all three
    stay fully SBUF-resident (no DRAM scratch roundtrip).
  - Scores are computed TRANSPOSED ([k, q] blocks): exp output feeds the
    P@V matmul directly as the stationary operand - no PE transposes.
  - softmax denominator = pe-block matmul against a ones column, giving
    [q-part, 1] psum, the right orientation for the final normalize
    (out = av * (1/den) + bv on DVE).
  - attention is software-pipelined one slot deep: scores(j+1) are issued
    before P@V(j) so the exp never stalls the tensor engine.
"""

from contextlib import ExitStack

import ml_dtypes
import numpy as np

import concourse.bacc as bacc
import concourse.mybir as mybir
import concourse.tile as tile
from concourse.bass_utils import run_bass_kernel_spmd

P = 128
D = 1024
S = 2048
B = 4
N_CORES = 8
EO = D // P            # 8 contraction chunks of 128
DO = D // P            # 8 output-dim chunks of 128
NT = S // P            # 16 k/s tiles of 128
NQ = 8                 # q-tile slots per core
F32 = mybir.dt.float32
BF16 = mybir.dt.bfloat16
NEG = -1.0e9
BF = ml_dtypes.bfloat16

_PROG_CACHE = {}


def _slot_gtiles(h, causal):
    """q-tile ids (units of 128 rows) owned by parity-h core, slot order."""
    if causal:
        return [2 * j + h for j in range(NQ)]
    return [8 * h + j for j in range(NQ)]


def build_program(causal: bool):
    nc = bacc.Bacc(trn_type="TRN2", target_bir_lowering=False, debug=False)

    def din(name, shape, dt=BF16):
        return nc.dram_tensor(name, shape, dt, kind="ExternalInput").ap()

    xq = din("xq", [P, EO, 1024])        # Xq^T for this core's 8 q-tiles
    xk = din("xk", [4, P, EO, 512])      # Xk^T, chunked along s
    xv = din("xv", [4, P, 4, EO, P])     # Xv^T, [group][p][s-tile][e][s]
    wq = din("wq", [P, EO, D])
    wk = din("wk", [P, EO, D])
    wv = din("wv", [P, EO, D])
    # bq | bk | bv | msk packed into one tensor -> one DMA
    cst = din("cst", [P, DO + DO + D + 2 * P], F32)
    out = nc.dram_tensor("out", [NQ, P, D], F32, kind="ExternalOutput").ap()

    Ident = mybir.ActivationFunctionType.Identity
    Exp = mybir.ActivationFunctionType.Exp
    add = mybir.AluOpType.add
    mult = mybir.AluOpType.mult

    # slot j processes nkt[j] k-tiles - identical on every core
    nkt = [2 * j + 2 if causal else NT for j in range(NQ)]

    with tile.TileContext(nc, pool_alloc_mode="queue") as tc, ExitStack() as top:
        const = top.enter_context(tc.tile_pool(name="const", bufs=1))
        cst_sb = const.tile([P, DO + DO + D + 2 * P], F32)
        nc.gpsimd.dma_start(out=cst_sb, in_=cst)
        bq_sb = cst_sb[:, 0:DO]
        bk_sb = cst_sb[:, DO:2 * DO]
        bv_sb = cst_sb[:, 2 * DO:2 * DO + D]
        msk_sb = cst_sb[:, 2 * DO + D:]
        ones_sb = const.tile([P, 1], BF16)
        nc.gpsimd.memset(ones_sb, 1.0)

        # resident projected tensors
        res = top.enter_context(tc.tile_pool(name="res", bufs=1))
        kt_sb = res.tile([P, DO, S], BF16, name="kt_sb")     # K^T [e, k]
        qt_sb = res.tile([P, DO, 1024], BF16, name="qt_sb")  # Q^T [e, q]
        v_sb = res.tile([P, NT, D], BF16, name="v_sb")       # V [s, d] blocked

        # ---------------- projections ----------------
        with tc.tile_pool(name="wt", bufs=2) as wtp, \
             tc.tile_pool(name="xin", bufs=3) as xinp, \
             tc.tile_pool(name="psA", bufs=3, space="PSUM") as psA, \
             tc.tile_pool(name="psB", bufs=2, space="PSUM") as psB:

            # K projection -> kt_sb [e-part, k], bias folded in.
            # DMA issues cost ~0.65us each on the issuing engine, so use
            # few, large transfers; split only the first-wave ones so the
            # first matmul doesn't wait for a full-tensor transfer.
            # first wave at per-eo granularity (latency); later transfers are
            # single big DMAs gated by tile-pool buffer reuse so they don't
            # steal DMA bandwidth from the startup-critical pieces
            qs = [nc.sync, nc.scalar, nc.gpsimd]
            wk_sb = wtp.tile([P, EO, D], BF16, tag="wt", name="wk_sb")
            xk_t0 = xinp.tile([P, EO, 1024], BF16, tag="xin", name="xk_t0")
            for eo in range(EO):
                qs[eo % 3].dma_start(out=wk_sb[:, eo, :], in_=wk[:, eo, :])
                qs[(eo + 1) % 3].dma_start(out=xk_t0[:, eo, 0:512],
                                           in_=xk[0, :, eo, :])
            for kc in range(4):
                if kc == 0:
                    xk_t = xk_t0
                else:
                    xk_t = xinp.tile([P, EO, 1024], BF16, tag="xin",
                                     name=f"xk_t{kc}")
                    nc.sync.dma_start(out=xk_t[:, :, 0:512], in_=xk[kc])
                for do in range(DO):
                    ps = psA.tile([P, 512], F32, tag="psA", name=f"psk{kc}_{do}")
                    for eo in range(EO):
                        nc.tensor.matmul(
                            ps,
                            lhsT=wk_sb[:, eo, do * P:(do + 1) * P],
                            rhs=xk_t[:, eo, 0:512],
                            start=(eo == 0), stop=(eo == EO - 1))
                    nc.scalar.activation(
                        out=kt_sb[:, do, kc * 512:(kc + 1) * 512], in_=ps,
                        func=Ident, bias=bk_sb[:, do:do + 1])

            # Q projection -> qt_sb [e-part, q], bias folded in
            wq_sb = wtp.tile([P, EO, D], BF16, tag="wt", name="wq_sb")
            xq_t = xinp.tile([P, EO, 1024], BF16, tag="xin", name="xq_t")
            nc.scalar.dma_start(out=wq_sb[:, 0:4, :], in_=wq[:, 0:4, :])
            nc.scalar.dma_start(out=wq_sb[:, 4:EO, :], in_=wq[:, 4:EO, :])
            nc.scalar.dma_start(out=xq_t, in_=xq)
            for sc in range(2):
                for do in range(DO):
                    ps = psA.tile([P, 512], F32, tag="psA", name=f"psq{sc}_{do}")
                    for eo in range(EO):
                        nc.tensor.matmul(
                            ps,
                            lhsT=wq_sb[:, eo, do * P:(do + 1) * P],
                            rhs=xq_t[:, eo, sc * 512:(sc + 1) * 512],
                            start=(eo == 0), stop=(eo == EO - 1))
                    nc.scalar.activation(
                        out=qt_sb[:, do, sc * 512:(sc + 1) * 512], in_=ps,
                        func=Ident, bias=bq_sb[:, do:do + 1])

            # V projection -> v_sb [s-part, d] blocked; bias folded at output
            wv_sb = wtp.tile([P, EO, D], BF16, tag="wt", name="wv_sb")
            nc.gpsimd.dma_start(out=wv_sb[:, 0:4, :], in_=wv[:, 0:4, :])
            nc.gpsimd.dma_start(out=wv_sb[:, 4:EO, :], in_=wv[:, 4:EO, :])
            for g4 in range(4):
                xv_t = xinp.tile([P, 4, EO, P], BF16, tag="xin",
                                 name=f"xv_t{g4}")
                eng = nc.sync if g4 % 2 == 0 else nc.gpsimd
                eng.dma_start(out=xv_t, in_=xv[g4])
                for sl in range(4):
                    st = 4 * g4 + sl
                    ps2 = psB.tile([P, D], F32, tag="psB", name=f"psv{st}")
                    for half in range(2):
                        for eo in range(EO):
                            nc.tensor.matmul(
                                ps2[:, half * 512:(half + 1) * 512],
                                lhsT=xv_t[:, sl, eo, :],
                                rhs=wv_sb[:, eo, half * 512:(half + 1) * 512],
                                start=(eo == 0), stop=(eo == EO - 1))
                    # evict on Act so DVE stays free for attention-phase work
                    nc.scalar.activation(out=v_sb[:, st, :], in_=ps2,
                                         func=mybir.ActivationFunctionType.Copy)

        # ---------------- attention, q-slot-major, 1-slot pipeline ----------
        with tc.tile_pool(name="pep", bufs=2) as pep, \
             tc.tile_pool(name="recp", bufs=2) as recp, \
             tc.tile_pool(name="outp", bufs=2) as outp, \
             tc.tile_pool(name="psS", bufs=3, space="PSUM") as psS, \
             tc.tile_pool(name="psV", bufs=2, space="PSUM") as psV, \
             tc.tile_pool(name="psD", bufs=1, space="PSUM") as psD:

            dn_ps = psD.tile([P, NQ], F32, tag="dn", name="dn_ps")
            pes = [None] * NQ

            def scores_slot(j):
                n_t = nkt[j]
                qc = slice(j * P, (j + 1) * P)
                pe = pep.tile([P, n_t * P], BF16, tag="pe", name=f"pe{j}")
                pes[j] = pe
                for c in range((n_t + 3) // 4):
                    t0 = 4 * c
                    w = min(4, n_t - t0) * P
                    ps = psS.tile([P, w], F32, tag="s", name=f"ps{j}_{c}")
                    for tl in range(w // P):
                        for do in range(DO):
                            nc.tensor.matmul(
                                ps[:, tl * P:(tl + 1) * P],
                                lhsT=kt_sb[:, do, (t0 + tl) * P:(t0 + tl + 1) * P],
                                rhs=qt_sb[:, do, qc],
                                start=(do == 0), stop=(do == DO - 1))
                    if causal and t0 + w // P == n_t:
                        # mask covers the last two k-tiles of the slot
                        nc.vector.tensor_tensor(
                            out=ps[:, w - 2 * P:w], in0=ps[:, w - 2 * P:w],
                            in1=msk_sb, op=add)
                    nc.scalar.activation(
                        out=pe[:, t0 * P:t0 * P + w], in_=ps, func=Exp,
                        scale=float(1.0 / np.sqrt(D)))

            def av_slot(j):
                n_t = nkt[j]
                pe = pes[j]
                av = psV.tile([P, D], F32, tag="av", name=f"av{j}")
                for t in range(n_t):
                    pblk = pe[:, t * P:(t + 1) * P]
                    nc.tensor.matmul(
                        dn_ps[:, j:j + 1], lhsT=pblk, rhs=ones_sb,
                        start=(t == 0), stop=(t == n_t - 1))
                    for half in range(2):
                        nc.tensor.matmul(
                            av[:, half * 512:(half + 1) * 512],
                            lhsT=pblk,
                            rhs=v_sb[:, t, half * 512:(half + 1) * 512],
                            start=(t == 0), stop=(t == n_t - 1))
                rec = recp.tile([P, 1], F32, tag="rec", name=f"rec{j}")
                nc.vector.reciprocal(out=rec, in_=dn_ps[:, j:j + 1])
                o = outp.tile([P, D], F32, tag="o", name=f"o{j}")
                for hf in range(2):
                    nc.vector.scalar_tensor_tensor(
                        out=o[:, hf * 512:(hf + 1) * 512],
                        in0=av[:, hf * 512:(hf + 1) * 512], scalar=rec,
                        in1=bv_sb[:, hf * 512:(hf + 1) * 512],
                        op0=mult, op1=add)
                    eng = nc.sync if hf == 0 else nc.scalar
                    eng.dma_start(out=out[j, :, hf * 512:(hf + 1) * 512],
                                  in_=o[:, hf * 512:(hf + 1) * 512])

            # big slots first: the kernel tail is the smallest slot's drain
            order = list(range(NQ - 1, -1, -1))
            for i, j in enumerate(order):
                scores_slot(j)
                if i > 0:
                    av_slot(order[i - 1])
            av_slot(order[-1])

    nc.compile()
    return nc


def _get_program(causal: bool):
    key = bool(causal)
    if key not in _PROG_CACHE:
        _PROG_CACHE[key] = build_program(key)
    return _PROG_CACHE[key]


def _shard_inputs(encoded_q, encoded_k, encoded_v, W_q, b_q, W_k, b_k,
                  W_v, b_v, causal):
    """Build the per-core in_maps (all host-side numpy, bf16 payloads)."""
    wqh = np.ascontiguousarray(
        W_q.T.reshape(EO, P, D).transpose(1, 0, 2)).astype(BF)
    wkh = np.ascontiguousarray(
        W_k.T.reshape(EO, P, D).transpose(1, 0, 2)).astype(BF)
    wvh = np.ascontiguousarray(
        W_v.T.reshape(EO, P, D).transpose(1, 0, 2)).astype(BF)
    bqh = np.ascontiguousarray(b_q.reshape(DO, P).T)
    bkh = np.ascontiguousarray(b_k.reshape(DO, P).T)
    bvh = np.ascontiguousarray(np.broadcast_to(b_v, (P, D)))

    ki = np.arange(P)[:, None]
    qi = np.arange(P)[None, :]
    tri = np.where(ki <= qi, 0.0, NEG).astype(np.float32)   # diagonal block
    zer = np.zeros((P, P), np.float32)
    ninf = np.full((P, P), NEG, np.float32)
    # h=0: slot j owns g=2j -> k-tile 2j is diagonal, 2j+1 fully masked
    # h=1: slot j owns g=2j+1 -> k-tile 2j unmasked, 2j+1 diagonal
    mskh = [np.concatenate([tri, ninf], 1), np.concatenate([zer, tri], 1)]

    in_maps = []
    for c in range(N_CORES):
        b, h = divmod(c, 2)
        gts = _slot_gtiles(h, causal)
        Xq = np.concatenate([encoded_q[b, g * P:(g + 1) * P, :] for g in gts], 0)
        xqh = np.ascontiguousarray(
            Xq.T.reshape(EO, P, 1024).transpose(1, 0, 2)).astype(BF)
        xkh = np.ascontiguousarray(
            encoded_k[b].T.reshape(EO, P, 4, 512).transpose(2, 1, 0, 3)).astype(BF)
        xvh = np.ascontiguousarray(
            encoded_v[b].T.reshape(EO, P, 4, 4, P)
            .transpose(2, 1, 3, 0, 4)).astype(BF)
        csth = np.ascontiguousarray(np.concatenate(
            [bqh, bkh, bvh,
             mskh[h] if causal else np.zeros((P, 2 * P), np.float32)], 1))
        in_maps.append({
            "xq": xqh, "xk": xkh, "xv": xvh,
            "wq": wqh, "wk": wkh, "wv": wvh, "cst": csth,
        })
    return in_maps


def kernel(encoded_q, encoded_k, encoded_v, W_q, b_q, W_k, b_k, W_v, b_v,
           parameter_mask, _want_trace=False, _trace_dir=None):
    causal = bool(np.asarray(parameter_mask).item())
    encoded_q = np.asarray(encoded_q, np.float32)
    encoded_k = np.asarray(encoded_k, np.float32)
    encoded_v = np.asarray(encoded_v, np.float32)
    nc = _get_program(causal)
    in_maps = _shard_inputs(encoded_q, encoded_k, encoded_v,
                            np.asarray(W_q, np.float32), np.asarray(b_q, np.float32),
                            np.asarray(W_k, np.float32), np.asarray(b_k, np.float32),
                            np.asarray(W_v, np.float32), np.asarray(b_v, np.float32),
                            causal)
    kw = {}
    if _want_trace:
        kw = dict(trace=True, tmpdir=_trace_dir)
    res = run_bass_kernel_spmd(nc, in_maps, core_ids=list(range(N_CORES)), **kw)

    full = np.empty((B, S, D), np.float32)
    for c in range(N_CORES):
        b, h = divmod(c, 2)
        o = res.results[c]["out"]
        for j, g in enumerate(_slot_gtiles(h, causal)):
            full[b, g * P:(g + 1) * P, :] = o[j]
    if _want_trace:
        return full, res
    return full
